# revision 1
# baseline (speedup 1.0000x reference)
"""Contrastive (NT-Xent) loss kernel for Trainium2, 8 NeuronCores SPMD.

Math (B=4096, D=256, T=0.5):
  z = l2norm(emb) rows; reps=[z_i; z_j] (8192 x 256); sim = reps @ reps.T
  denom_r = sum_{c != r} exp(sim[r,c]/T);  pos_m = z_i[m].z_j[m]
  loss = mean_r( ln(denom_r) - pos_r/T )

Per-core plan (core k owns reps rows: z_i rows [512k,512k+512) and z_j rows
[512k,512k+512) -> 8 m-tiles of 128):
  - load full emb_i/emb_j (replicated) + own row blocks (sharded)
  - rowwise sq-sums (DVE ttr), inv_norm = Exp(-0.5*Ln(s)) (ACT, same table
    set as the main exp/ln), normalize to fp16 (DVE tensor_scalar)
  - transpose to d-major zT [128d x cols] via DMA xbar (fp16, SBUF->SBUF)
  - for each 2048-col group g, m-tile: matmul fp16 -> PSUM fp32 [128,2048],
    ACT Exp(scale=2) in-place with accum_out -> per-row partial sums
  - rowsum -> ln(rowsum - e^2) (removes the diag term exactly enough),
    minus 4*sum(pos), -> per-partition partial [128,1] per core
Host: loss = sum(partials)/(2B).  (gather/unshard = sum of shards)
"""

import os
import numpy as np
from contextlib import ExitStack

import concourse.bass as bass
import concourse.tile as tile
from concourse import bacc, mybir
from concourse import bass_utils

B = 4096
D = 256
TEMP = 0.5
NCORES = 8
ROWS = 2 * B            # 8192 reps rows
PER = B // NCORES       # 512 rows of emb_i (and emb_j) per core
OWN = 2 * PER           # 1024 reps rows per core
P = 128
NG = 4                  # column groups
GCOLS = ROWS // NG      # 2048 columns per group
MT = OWN // P           # 8 m-tiles per core
F32 = mybir.dt.float32
DT = mybir.dt.float16   # matmul input dtype
INV_T = 1.0 / TEMP      # 2.0
DIAG = float(np.exp(np.float32(INV_T), dtype=np.float32))  # exp(2*||z||^2), ||z||~1

# "pe" = TensorE transpose + DVE evac (PSUM shared with matmul groups)
# "dma" = DMA xbar transpose SBUF->SBUF
TRANSPOSE_MODE = os.environ.get("CL_TRANSPOSE", "dma")


def _kernel_body(ctx: ExitStack, tc: tile.TileContext, out_ap, xi, xj, oa, ob):
    nc = tc.nc
    AF = mybir.ActivationFunctionType
    ALU = mybir.AluOpType

    x_pool = ctx.enter_context(tc.tile_pool(name="x", bufs=4))
    z_pool = ctx.enter_context(tc.tile_pool(name="z", bufs=4))
    zt_pool = ctx.enter_context(tc.tile_pool(name="zt", bufs=1))
    own_pool = ctx.enter_context(tc.tile_pool(name="own", bufs=1))
    st_pool = ctx.enter_context(tc.tile_pool(name="st", bufs=2))
    fin_pool = ctx.enter_context(tc.tile_pool(name="fin", bufs=1))
    ps_pool = ctx.enter_context(tc.tile_pool(name="ps", bufs=2, space="PSUM"))

    dummy = fin_pool.tile([P, 1], F32, tag="dummy")
    rowparts = fin_pool.tile([P, MT * NG], F32, tag="rowparts")
    negdiag = fin_pool.tile([P, 1], F32, tag="negdiag")
    nc.gpsimd.memset(negdiag[:], -DIAG)

    if TRANSPOSE_MODE == "pe":
        ident = fin_pool.tile([P, P], DT, tag="ident")
        from concourse.masks import make_identity
        make_identity(nc, ident[:])

    sq_pool = ctx.enter_context(tc.tile_pool(name="sq", bufs=2))

    def sqsum_x3(x3, nt, sqs_ap):
        # sqs_ap[p, t] = sum_d x3[p,t,d]^2  (one big mul + one 3D reduce)
        sq3 = sq_pool.tile([P, nt, D], F32, tag="sq3", name="sq3")
        nc.vector.tensor_mul(sq3[:], x3[:, 0:nt, :], x3[:, 0:nt, :])
        nc.vector.reduce_sum(out=sqs_ap, in_=sq3[:], axis=mybir.AxisListType.X)

    def inv_from_sqs(sqs_ap, inv_ap):
        # inv = s^-0.5 = Exp(-0.5*Ln(s)); Ln+Exp live in one ACT table set
        nc.scalar.activation(out=inv_ap, in_=sqs_ap, func=AF.Ln)
        nc.scalar.activation(out=inv_ap, in_=inv_ap, func=AF.Exp, scale=-0.5)

    def transpose_block(zt_tile, col0, z3, t, h):
        # zt_tile[:, col0:col0+128] = z3[:, t, h*128:(h+1)*128].T
        src = z3[:, t, h * P:(h + 1) * P]
        dst = zt_tile[:, col0:col0 + P]
        nc.sync.dma_start_transpose(out=dst, in_=src)

    def transpose_group(zt_lo, zt_hi, z3s, nt):
        # z3s: list of (z3, local_t) covering nt row-tiles in column order
        if TRANSPOSE_MODE == "pe":
            for h, zt_t in ((0, zt_lo), (1, zt_hi)):
                ps = ps_pool.tile([P, GCOLS], DT, tag="ps", name="ps_tr")
                for u, (z3, t) in enumerate(z3s):
                    nc.tensor.transpose(
                        ps[:, u * P:(u + 1) * P], z3[:, t, h * P:(h + 1) * P],
                        ident[:])
                nc.vector.tensor_copy(zt_t[:, 0:nt * P], ps[:, 0:nt * P])
        else:
            for h, zt_t in ((0, zt_lo), (1, zt_hi)):
                for u, (z3, t) in enumerate(z3s):
                    transpose_block(zt_t, u * P, z3, t, h)

    # ---------------- own-block prologue ----------------
    own_x = own_pool.tile([P, 2 * (PER // P), D], F32, tag="own_x")  # [128,8,256]
    nt_own = PER // P  # 4
    nc.sync.dma_start(own_x[:, 0:nt_own, :], oa.rearrange("(t p) d -> p t d", p=P))
    nc.sync.dma_start(own_x[:, nt_own:2 * nt_own, :], ob.rearrange("(t p) d -> p t d", p=P))

    sqs_own = own_pool.tile([P, 2 * nt_own], F32, tag="sqs_own")
    sqsum_x3(own_x, 2 * nt_own, sqs_own[:])
    inv_own = own_pool.tile([P, 2 * nt_own], F32, tag="inv_own")
    inv_from_sqs(sqs_own[:], inv_own[:])

    z_own = own_pool.tile([P, 2 * nt_own, D], DT, tag="z_own")
    for t in range(2 * nt_own):
        nc.vector.tensor_scalar_mul(
            out=z_own[:, t, :], in0=own_x[:, t, :], scalar1=inv_own[:, t:t + 1])

    zt_own = [own_pool.tile([P, OWN], DT, tag=f"zt_own{h}", name=f"zt_own{h}")
              for h in range(2)]
    transpose_group(zt_own[0], zt_own[1],
                    [(z_own, t) for t in range(2 * nt_own)], 2 * nt_own)

    # positives: pos_t = (x_a[t] . x_b[t]) * inv_a[t] * inv_b[t]
    pos_raw = own_pool.tile([P, nt_own], F32, tag="pos_raw")
    pr3 = sq_pool.tile([P, nt_own, D], F32, tag="sq3", name="pr3")
    nc.vector.tensor_mul(pr3[:], own_x[:, 0:nt_own, :], own_x[:, nt_own:2 * nt_own, :])
    nc.vector.reduce_sum(out=pos_raw[:], in_=pr3[:], axis=mybir.AxisListType.X)
    pos = own_pool.tile([P, nt_own], F32, tag="pos")
    nc.vector.tensor_mul(pos[:], pos_raw[:], inv_own[:, 0:nt_own])
    nc.vector.tensor_mul(pos[:], pos[:], inv_own[:, nt_own:2 * nt_own])

    # ---------------- full-rep group prologue ----------------
    zt = [[None, None] for _ in range(NG)]

    def prologue_group(g):
        src = xi if g < 2 else xj
        r0 = (g % 2) * GCOLS
        nt = GCOLS // P  # 16 row-tiles
        x3s = []
        for half in range(2):
            x3 = x_pool.tile([P, 8, D], F32, tag="x")
            rows = src[r0 + half * 1024: r0 + (half + 1) * 1024]
            nc.sync.dma_start(x3[:], rows.rearrange("(t p) d -> p t d", p=P))
            x3s.append(x3)
        sqs = st_pool.tile([P, nt], F32, tag="sqs")
        for half in range(2):
            sqsum_x3(x3s[half], 8, sqs[:, half * 8:(half + 1) * 8])
        inv = st_pool.tile([P, nt], F32, tag="inv")
        inv_from_sqs(sqs[:], inv[:])
        z3s = []
        for half in range(2):
            z3 = z_pool.tile([P, 8, D], DT, tag="z")
            for t in range(8):
                nc.vector.tensor_scalar_mul(
                    out=z3[:, t, :], in0=x3s[half][:, t, :],
                    scalar1=inv[:, half * 8 + t: half * 8 + t + 1])
            z3s.append(z3)
        zt[g][0] = zt_pool.tile([P, GCOLS], DT, tag=f"zt{g}_0", name=f"zt{g}_0")
        zt[g][1] = zt_pool.tile([P, GCOLS], DT, tag=f"zt{g}_1", name=f"zt{g}_1")
        transpose_group(zt[g][0], zt[g][1],
                        [(z3s[t // 8], t % 8) for t in range(nt)], nt)

    def main_unit(g, m):
        ps = ps_pool.tile([P, GCOLS], F32, tag="ps")
        nsub = GCOLS // 512
        for ns in range(nsub):
            nc.tensor.matmul(
                ps[:, ns * 512:(ns + 1) * 512],
                lhsT=zt_own[0][:, m * P:(m + 1) * P],
                rhs=zt[g][0][:, ns * 512:(ns + 1) * 512],
                start=True, stop=False)
        for ns in range(nsub):
            nc.tensor.matmul(
                ps[:, ns * 512:(ns + 1) * 512],
                lhsT=zt_own[1][:, m * P:(m + 1) * P],
                rhs=zt[g][1][:, ns * 512:(ns + 1) * 512],
                start=False, stop=True)
        nc.scalar.activation(
            out=ps[:], in_=ps[:], func=AF.Exp, scale=INV_T,
            accum_out=rowparts[:, m * NG + g: m * NG + g + 1])

    prologue_group(0)
    for g in range(NG):
        for m in range(MT // 2):
            main_unit(g, m)
        if g + 1 < NG:
            prologue_group(g + 1)
        for m in range(MT // 2, MT):
            main_unit(g, m)

    # ---------------- tail ----------------
    denom = fin_pool.tile([P, MT], F32, tag="denom")
    nc.vector.reduce_sum(
        out=denom[:], in_=rowparts[:].rearrange("p (m g) -> p m g", g=NG),
        axis=mybir.AxisListType.X)
    ln8 = fin_pool.tile([P, MT], F32, tag="ln8")
    nc.scalar.activation(out=ln8[:], in_=denom[:], func=AF.Ln, bias=negdiag[:])
    lnsum = fin_pool.tile([P, 1], F32, tag="lnsum")
    nc.vector.reduce_sum(out=lnsum[:], in_=ln8[:], axis=mybir.AxisListType.X)
    possum = fin_pool.tile([P, 1], F32, tag="possum")
    nc.vector.reduce_sum(out=possum[:], in_=pos[:], axis=mybir.AxisListType.X)
    partial = fin_pool.tile([P, 1], F32, tag="partial")
    # partial = lnsum - 2*INV_T*possum   (each pos appears for a z_i and a z_j row)
    nc.vector.tensor_scalar(
        out=partial[:], in0=possum[:], scalar1=-2.0 * INV_T, scalar2=lnsum[:],
        op0=ALU.mult, op1=ALU.add)
    nc.sync.dma_start(out_ap, partial[:])


_NC_CACHE = {}


def build_nc():
    key = TRANSPOSE_MODE
    if key in _NC_CACHE:
        return _NC_CACHE[key]
    nc = bacc.Bacc("TRN2", target_bir_lowering=False, debug=False,
                   enable_asserts=False, num_devices=NCORES)
    xi = nc.dram_tensor("xi", (B, D), F32, kind="ExternalInput").ap()
    xj = nc.dram_tensor("xj", (B, D), F32, kind="ExternalInput").ap()
    oa = nc.dram_tensor("oa", (PER, D), F32, kind="ExternalInput").ap()
    ob = nc.dram_tensor("ob", (PER, D), F32, kind="ExternalInput").ap()
    out = nc.dram_tensor("out", (P, 1), F32, kind="ExternalOutput").ap()
    with tile.TileContext(nc) as tc:
        with ExitStack() as ctx:
            _kernel_body(ctx, tc, out, xi, xj, oa, ob)
    nc.compile()
    _NC_CACHE[key] = nc
    return nc


def make_in_maps(emb_i, emb_j):
    emb_i = np.ascontiguousarray(np.asarray(emb_i, dtype=np.float32))
    emb_j = np.ascontiguousarray(np.asarray(emb_j, dtype=np.float32))
    maps = []
    for k in range(NCORES):
        maps.append({
            "xi": emb_i,
            "xj": emb_j,
            "oa": np.ascontiguousarray(emb_i[k * PER:(k + 1) * PER]),
            "ob": np.ascontiguousarray(emb_j[k * PER:(k + 1) * PER]),
        })
    return maps


def run(emb_i, emb_j, trace=False, **kw):
    nc = build_nc()
    res = bass_utils.run_bass_kernel_spmd(
        nc, make_in_maps(emb_i, emb_j), core_ids=list(range(NCORES)),
        trace=trace, **kw)
    partials = np.stack([r["out"] for r in res.results])  # [8,128,1]
    loss = np.float32(partials.astype(np.float64).sum() / ROWS)
    return loss, res


def kernel(emb_i, emb_j):
    loss, _ = run(emb_i, emb_j, trace=False)
    return np.asarray(loss, dtype=np.float32)



# revision 2
# speedup vs baseline: 4.5760x; 4.5760x over previous
"""Contrastive (NT-Xent) loss kernel for Trainium2, 8 NeuronCores SPMD.

Math (B=4096, D=256, T=0.5):
  z = l2norm(emb) rows; reps=[z_i; z_j] (8192 x 256); sim = reps @ reps.T
  denom_r = sum_{c != r} exp(sim[r,c]/T);  pos_m = z_i[m].z_j[m]
  loss = mean_r( ln(denom_r) - pos_r/T )

Distribution: core k receives ONLY its row shard x = [emb_i rows
[512k,512k+512); emb_j rows [512k,512k+512)] as fp16 (512KB/core instead
of a replicated 9MB) — H2D over the axon tunnel is the wall-clock
bottleneck, not device compute. Each core normalizes its 1024 rows,
transposes them to d-major fp16 tiles, and the 8 cores AllGather those
tiles HBM->HBM (512KB -> 4MB, ~15us on-chip, compute engines stay free).
The gathered column order is a core-major permutation of the reference
row order, which is harmless: the denominator is a permutation-invariant
row sum and the diagonal term is removed analytically (exp(2*||z||^2)=e^2).

Per-core main loop (8 m-tiles x 4 column groups of 2048):
  matmul fp16 -> PSUM fp32 [128,2048], ACT Exp(scale=2) in-place with
  accum_out -> per-(m,g) row partial sums; tail: ln(rowsum - e^2) minus
  4*sum(pos) -> per-partition partial [128,1] per core.
Host: loss = sum(partials)/(2B).  (gather/unshard = sum of shards)
"""

import numpy as np
from contextlib import ExitStack

import concourse.bass as bass
import concourse.tile as tile
from concourse import bacc, mybir
from concourse import bass_utils

B = 4096
D = 256
TEMP = 0.5
NCORES = 8
ROWS = 2 * B            # 8192 reps rows
PER = B // NCORES       # 512 rows of emb_i (and emb_j) per core
OWN = 2 * PER           # 1024 reps rows per core
P = 128
NG = 4                  # column groups
GCOLS = ROWS // NG      # 2048 columns per group
MT = OWN // P           # 8 m-tiles per core
NT = OWN // P           # 8 own row-tiles
F32 = mybir.dt.float32
DT = mybir.dt.float16   # wire + matmul dtype
INV_T = 1.0 / TEMP      # 2.0
DIAG = float(np.exp(np.float32(INV_T), dtype=np.float32))  # exp(2*||z||^2), ||z||~1


def _kernel_body(ctx: ExitStack, tc: tile.TileContext, out_ap, x):
    nc = tc.nc
    AF = mybir.ActivationFunctionType
    ALU = mybir.AluOpType

    own_pool = ctx.enter_context(tc.tile_pool(name="own", bufs=1))
    zt_pool = ctx.enter_context(tc.tile_pool(name="zt", bufs=1))
    fin_pool = ctx.enter_context(tc.tile_pool(name="fin", bufs=1))
    ps_pool = ctx.enter_context(tc.tile_pool(name="ps", bufs=2, space="PSUM"))
    dram_pool = ctx.enter_context(tc.tile_pool(name="dram", bufs=1, space="DRAM"))

    rowparts = fin_pool.tile([P, MT * NG], F32, tag="rowparts")
    negdiag = fin_pool.tile([P, 1], F32, tag="negdiag")
    nc.gpsimd.memset(negdiag[:], -DIAG)

    # ---------------- own-block prologue ----------------
    own_x = own_pool.tile([P, NT, D], DT, tag="own_x")  # [128,8,256]
    nc.sync.dma_start(own_x[:], x.rearrange("(t p) d -> p t d", p=P))

    # rowwise sq-sums -> inv_norm = Exp(-0.5*Ln(s))
    sq3 = own_pool.tile([P, NT, D], F32, tag="sq3")
    nc.vector.tensor_mul(sq3[:], own_x[:], own_x[:])
    sqs = own_pool.tile([P, NT], F32, tag="sqs")
    nc.vector.reduce_sum(out=sqs[:], in_=sq3[:], axis=mybir.AxisListType.X)
    inv = own_pool.tile([P, NT], F32, tag="inv")
    nc.scalar.activation(out=inv[:], in_=sqs[:], func=AF.Ln)
    nc.scalar.activation(out=inv[:], in_=inv[:], func=AF.Exp, scale=-0.5)

    z_own = own_pool.tile([P, NT, D], DT, tag="z_own")
    for t in range(NT):
        nc.vector.tensor_scalar_mul(
            out=z_own[:, t, :], in0=own_x[:, t, :], scalar1=inv[:, t:t + 1])

    # transpose own rows to d-major: zt_own[h][d, col] with d in half h
    zt_own = [own_pool.tile([P, OWN], DT, tag=f"zt_own{h}", name=f"zt_own{h}")
              for h in range(2)]
    for h in range(2):
        for t in range(NT):
            nc.sync.dma_start_transpose(
                out=zt_own[h][:, t * P:(t + 1) * P],
                in_=z_own[:, t, h * P:(h + 1) * P])

    # positives: pos_t = (x_i[t] . x_j[t]) * inv_i[t] * inv_j[t]
    nt2 = NT // 2
    pr3 = own_pool.tile([P, nt2, D], F32, tag="pr3")
    nc.vector.tensor_mul(pr3[:], own_x[:, 0:nt2, :], own_x[:, nt2:NT, :])
    pos = own_pool.tile([P, nt2], F32, tag="pos")
    nc.vector.reduce_sum(out=pos[:], in_=pr3[:], axis=mybir.AxisListType.X)
    nc.vector.tensor_mul(pos[:], pos[:], inv[:, 0:nt2])
    nc.vector.tensor_mul(pos[:], pos[:], inv[:, nt2:NT])

    # ---------------- all-gather d-major z ----------------
    cc_in = dram_pool.tile([2, P, OWN], DT, name="cc_in")
    for h in range(2):
        nc.gpsimd.dma_start(cc_in[h], zt_own[h][:])
    cc_out = dram_pool.tile([NCORES, 2, P, OWN], DT, addr_space="Shared",
                            name="cc_out")
    nc.gpsimd.collective_compute(
        "AllGather", mybir.AluOpType.bypass,
        replica_groups=[list(range(NCORES))],
        ins=[cc_in.opt()], outs=[cc_out.opt()])

    # rhs tiles: zt[g][h][:, j*OWN:(j+1)*OWN] = core (2g+j)'s half-h block
    zt = [[None, None] for _ in range(NG)]
    for g in range(NG):
        for h in range(2):
            zt[g][h] = zt_pool.tile([P, GCOLS], DT, tag=f"zt{g}_{h}",
                                    name=f"zt{g}_{h}")
            for j in range(2):
                nc.sync.dma_start(zt[g][h][:, j * OWN:(j + 1) * OWN],
                                  cc_out[2 * g + j, h])

    # ---------------- main loop ----------------
    def main_unit(g, m):
        ps = ps_pool.tile([P, GCOLS], F32, tag="ps", name="ps")
        nsub = GCOLS // 512
        for h in range(2):
            for ns in range(nsub):
                nc.tensor.matmul(
                    ps[:, ns * 512:(ns + 1) * 512],
                    lhsT=zt_own[h][:, m * P:(m + 1) * P],
                    rhs=zt[g][h][:, ns * 512:(ns + 1) * 512],
                    start=(h == 0), stop=(h == 1))
        nc.scalar.activation(
            out=ps[:], in_=ps[:], func=AF.Exp, scale=INV_T,
            accum_out=rowparts[:, m * NG + g: m * NG + g + 1])

    for g in range(NG):
        for m in range(MT):
            main_unit(g, m)

    # ---------------- tail ----------------
    denom = fin_pool.tile([P, MT], F32, tag="denom")
    nc.vector.reduce_sum(
        out=denom[:], in_=rowparts[:].rearrange("p (m g) -> p m g", g=NG),
        axis=mybir.AxisListType.X)
    ln8 = fin_pool.tile([P, MT], F32, tag="ln8")
    nc.scalar.activation(out=ln8[:], in_=denom[:], func=AF.Ln, bias=negdiag[:])
    lnsum = fin_pool.tile([P, 1], F32, tag="lnsum")
    nc.vector.reduce_sum(out=lnsum[:], in_=ln8[:], axis=mybir.AxisListType.X)
    possum = fin_pool.tile([P, 1], F32, tag="possum")
    nc.vector.reduce_sum(out=possum[:], in_=pos[:], axis=mybir.AxisListType.X)
    partial = fin_pool.tile([P, 1], F32, tag="partial")
    # partial = lnsum - 2*INV_T*possum   (each pos appears for a z_i and a z_j row)
    nc.vector.tensor_scalar(
        out=partial[:], in0=possum[:], scalar1=-2.0 * INV_T, scalar2=lnsum[:],
        op0=ALU.mult, op1=ALU.add)
    nc.sync.dma_start(out_ap, partial[:])


_NC_CACHE = {}


def build_nc():
    if "nc" in _NC_CACHE:
        return _NC_CACHE["nc"]
    nc = bacc.Bacc("TRN2", target_bir_lowering=False, debug=False,
                   enable_asserts=False, num_devices=NCORES)
    x = nc.dram_tensor("x", (OWN, D), DT, kind="ExternalInput").ap()
    out = nc.dram_tensor("out", (P, 1), F32, kind="ExternalOutput").ap()
    with tile.TileContext(nc) as tc:
        with ExitStack() as ctx:
            _kernel_body(ctx, tc, out, x)
    nc.compile()
    _NC_CACHE["nc"] = nc
    return nc


def make_in_maps(emb_i, emb_j):
    xi = np.asarray(emb_i, dtype=np.float16).reshape(NCORES, PER, D)
    xj = np.asarray(emb_j, dtype=np.float16).reshape(NCORES, PER, D)
    xall = np.concatenate([xi, xj], axis=1)  # [8, 1024, 256]
    return [{"x": xall[k]} for k in range(NCORES)]


def run(emb_i, emb_j, trace=False, **kw):
    nc = build_nc()
    res = bass_utils.run_bass_kernel_spmd(
        nc, make_in_maps(emb_i, emb_j), core_ids=list(range(NCORES)),
        trace=trace, **kw)
    partials = np.stack([r["out"] for r in res.results])  # [8,128,1]
    loss = np.float32(partials.astype(np.float64).sum() / ROWS)
    return loss, res


def kernel(emb_i, emb_j):
    loss, _ = run(emb_i, emb_j, trace=False)
    return np.asarray(loss, dtype=np.float32)


# revision 3
# speedup vs baseline: 8.3314x; 1.8207x over previous
"""Contrastive (NT-Xent) loss kernel for Trainium2, 8 NeuronCores SPMD.

Math (B=4096, D=256, T=0.5):
  z = l2norm(emb) rows; reps=[z_i; z_j] (8192 x 256); sim = reps @ reps.T
  denom_r = sum_{c != r} exp(sim[r,c]/T);  pos_m = z_i[m].z_j[m]
  loss = mean_r( ln(denom_r) - pos_r/T )

Distribution: core k receives ONLY its row shard x = [emb_i rows
[512k,512k+512); emb_j rows [512k,512k+512)] as fp16 (512KB/core instead
of a replicated 9MB) — H2D over the axon tunnel is the wall-clock
bottleneck, not device compute. Each core normalizes its 1024 rows,
transposes them to d-major fp16 tiles, and the 8 cores AllGather those
tiles HBM->HBM (512KB -> 4MB, ~15us on-chip, compute engines stay free).
The gathered column order is a core-major permutation of the reference
row order, which is harmless: the denominator is a permutation-invariant
row sum and the diagonal term is removed analytically (exp(2*||z||^2)=e^2).

Per-core main loop (8 m-tiles x 4 column groups of 2048):
  matmul fp16 -> PSUM fp32 [128,2048], ACT Exp(scale=2) in-place with
  accum_out -> per-(m,g) row partial sums; tail: ln(rowsum - e^2) minus
  4*sum(pos) -> per-partition partial [128,1] per core.
Host: loss = sum(partials)/(2B).  (gather/unshard = sum of shards)
"""

import numpy as np
from contextlib import ExitStack

import concourse.bass as bass
import concourse.tile as tile
from concourse import bacc, mybir
from concourse import bass_utils

B = 4096
D = 256
TEMP = 0.5
NCORES = 8
ROWS = 2 * B            # 8192 reps rows
PER = B // NCORES       # 512 rows of emb_i (and emb_j) per core
OWN = 2 * PER           # 1024 reps rows per core
P = 128
NG = 4                  # column groups
GCOLS = ROWS // NG      # 2048 columns per group
MT = OWN // P           # 8 m-tiles per core
NT = OWN // P           # 8 own row-tiles
F32 = mybir.dt.float32
DT = mybir.dt.float16   # wire + matmul dtype
INV_T = 1.0 / TEMP      # 2.0
DIAG = float(np.exp(np.float32(INV_T), dtype=np.float32))  # exp(2*||z||^2), ||z||~1


def _kernel_body(ctx: ExitStack, tc: tile.TileContext, out_ap, x):
    nc = tc.nc
    AF = mybir.ActivationFunctionType
    ALU = mybir.AluOpType

    own_pool = ctx.enter_context(tc.tile_pool(name="own", bufs=1))
    zt_pool = ctx.enter_context(tc.tile_pool(name="zt", bufs=1))
    fin_pool = ctx.enter_context(tc.tile_pool(name="fin", bufs=1))
    ps_pool = ctx.enter_context(tc.tile_pool(name="ps", bufs=2, space="PSUM"))
    dram_pool = ctx.enter_context(tc.tile_pool(name="dram", bufs=1, space="DRAM"))

    rowparts = fin_pool.tile([P, MT * NG], F32, tag="rowparts")
    negdiag = fin_pool.tile([P, 1], F32, tag="negdiag")
    nc.gpsimd.memset(negdiag[:], -DIAG)

    # ---------------- own-block prologue ----------------
    own_x = own_pool.tile([P, NT, D], DT, tag="own_x")  # [128,8,256]
    nc.sync.dma_start(own_x[:], x.rearrange("(t p) d -> p t d", p=P))

    # rowwise sq-sums -> inv_norm = Exp(-0.5*Ln(s))
    sq3 = own_pool.tile([P, NT, D], F32, tag="sq3")
    nc.vector.tensor_mul(sq3[:], own_x[:], own_x[:])
    sqs = own_pool.tile([P, NT], F32, tag="sqs")
    nc.vector.reduce_sum(out=sqs[:], in_=sq3[:], axis=mybir.AxisListType.X)
    inv = own_pool.tile([P, NT], F32, tag="inv")
    nc.scalar.activation(out=inv[:], in_=sqs[:], func=AF.Ln)
    nc.scalar.activation(out=inv[:], in_=inv[:], func=AF.Exp, scale=-0.5)

    z_own = own_pool.tile([P, NT, D], DT, tag="z_own")
    for t in range(NT):
        nc.vector.tensor_scalar_mul(
            out=z_own[:, t, :], in0=own_x[:, t, :], scalar1=inv[:, t:t + 1])

    # transpose own rows to d-major: zt_own[h][d, col] with d in half h
    zt_own = [own_pool.tile([P, OWN], DT, tag=f"zt_own{h}", name=f"zt_own{h}")
              for h in range(2)]
    for h in range(2):
        for t in range(NT):
            nc.sync.dma_start_transpose(
                out=zt_own[h][:, t * P:(t + 1) * P],
                in_=z_own[:, t, h * P:(h + 1) * P])

    # positives: pos_t = (x_i[t] . x_j[t]) * inv_i[t] * inv_j[t]
    nt2 = NT // 2
    pr3 = own_pool.tile([P, nt2, D], F32, tag="pr3")
    nc.vector.tensor_mul(pr3[:], own_x[:, 0:nt2, :], own_x[:, nt2:NT, :])
    pos = own_pool.tile([P, nt2], F32, tag="pos")
    nc.vector.reduce_sum(out=pos[:], in_=pr3[:], axis=mybir.AxisListType.X)
    nc.vector.tensor_mul(pos[:], pos[:], inv[:, 0:nt2])
    nc.vector.tensor_mul(pos[:], pos[:], inv[:, nt2:NT])

    # ---------------- all-gather d-major z ----------------
    cc_in = dram_pool.tile([2, P, OWN], DT, name="cc_in")
    for h in range(2):
        nc.gpsimd.dma_start(cc_in[h], zt_own[h][:])
    cc_out = dram_pool.tile([NCORES, 2, P, OWN], DT, addr_space="Shared",
                            name="cc_out")
    nc.gpsimd.collective_compute(
        "AllGather", mybir.AluOpType.bypass,
        replica_groups=[list(range(NCORES))],
        ins=[cc_in.opt()], outs=[cc_out.opt()])

    # rhs tiles: zt[g][h][:, j*OWN:(j+1)*OWN] = core (2g+j)'s half-h block
    zt = [[None, None] for _ in range(NG)]
    for g in range(NG):
        for h in range(2):
            zt[g][h] = zt_pool.tile([P, GCOLS], DT, tag=f"zt{g}_{h}",
                                    name=f"zt{g}_{h}")
            for j in range(2):
                nc.sync.dma_start(zt[g][h][:, j * OWN:(j + 1) * OWN],
                                  cc_out[2 * g + j, h])

    # ---------------- main loop ----------------
    def main_unit(g, m):
        ps = ps_pool.tile([P, GCOLS], F32, tag="ps", name="ps")
        nsub = GCOLS // 512
        for h in range(2):
            for ns in range(nsub):
                nc.tensor.matmul(
                    ps[:, ns * 512:(ns + 1) * 512],
                    lhsT=zt_own[h][:, m * P:(m + 1) * P],
                    rhs=zt[g][h][:, ns * 512:(ns + 1) * 512],
                    start=(h == 0), stop=(h == 1))
        nc.scalar.activation(
            out=ps[:], in_=ps[:], func=AF.Exp, scale=INV_T,
            accum_out=rowparts[:, m * NG + g: m * NG + g + 1])

    for g in range(NG):
        for m in range(MT):
            main_unit(g, m)

    # ---------------- tail ----------------
    denom = fin_pool.tile([P, MT], F32, tag="denom")
    nc.vector.reduce_sum(
        out=denom[:], in_=rowparts[:].rearrange("p (m g) -> p m g", g=NG),
        axis=mybir.AxisListType.X)
    ln8 = fin_pool.tile([P, MT], F32, tag="ln8")
    nc.scalar.activation(out=ln8[:], in_=denom[:], func=AF.Ln, bias=negdiag[:])
    lnsum = fin_pool.tile([P, 1], F32, tag="lnsum")
    nc.vector.reduce_sum(out=lnsum[:], in_=ln8[:], axis=mybir.AxisListType.X)
    possum = fin_pool.tile([P, 1], F32, tag="possum")
    nc.vector.reduce_sum(out=possum[:], in_=pos[:], axis=mybir.AxisListType.X)
    partial = fin_pool.tile([P, 1], F32, tag="partial")
    # partial = lnsum - 2*INV_T*possum   (each pos appears for a z_i and a z_j row)
    nc.vector.tensor_scalar(
        out=partial[:], in0=possum[:], scalar1=-2.0 * INV_T, scalar2=lnsum[:],
        op0=ALU.mult, op1=ALU.add)
    nc.sync.dma_start(out_ap, partial[:])


_NC_CACHE = {}


def build_nc():
    if "nc" in _NC_CACHE:
        return _NC_CACHE["nc"]
    nc = bacc.Bacc("TRN2", target_bir_lowering=False, debug=False,
                   enable_asserts=False, num_devices=NCORES)
    x = nc.dram_tensor("x", (OWN, D), DT, kind="ExternalInput").ap()
    out = nc.dram_tensor("out", (P, 1), F32, kind="ExternalOutput").ap()
    with tile.TileContext(nc) as tc:
        with ExitStack() as ctx:
            _kernel_body(ctx, tc, out, x)
    nc.compile()
    _NC_CACHE["nc"] = nc
    return nc


def _make_xall(emb_i, emb_j):
    xi = np.asarray(emb_i, dtype=np.float16).reshape(NCORES, PER, D)
    xj = np.asarray(emb_j, dtype=np.float16).reshape(NCORES, PER, D)
    return np.concatenate([xi, xj], axis=1)  # [8, 1024, 256]; [k] = core k shard


def make_in_maps(emb_i, emb_j):
    xall = _make_xall(emb_i, emb_j)
    return [{"x": xall[k]} for k in range(NCORES)]


def _build_fast_call(nc):
    """Cached-dispatch twin of bass_utils.run_bass_kernel_spmd's axon/PJRT
    exec step: same NEFF, same shard_map over cores 0-7, but the jitted
    callable is built once and reused, instead of a fresh closure (and a
    ~0.15s jax re-trace) per call."""
    import jax
    from concourse.bass2jax import (_bass_exec_p, install_neuronx_cc_hook,
                                    partition_id_tensor)
    from jax.sharding import Mesh, PartitionSpec
    from jax.experimental.shard_map import shard_map

    install_neuronx_cc_hook()
    partition_name = (nc.partition_id_tensor.name
                      if nc.partition_id_tensor else None)
    in_names, out_names, out_avals = [], [], []
    for alloc in nc.m.functions[0].allocations:
        if not isinstance(alloc, mybir.MemoryLocationSet):
            continue
        name = alloc.memorylocations[0].name
        if alloc.kind == "ExternalInput":
            if name != partition_name:
                in_names.append(name)
        elif alloc.kind == "ExternalOutput":
            out_names.append(name)
            shape = tuple(alloc.tensor_shape)
            out_avals.append(jax.core.ShapedArray(shape, mybir.dt.np(alloc.dtype)))
    assert in_names == ["x"] and out_names == ["out"]
    n_params = len(in_names)
    in_names.extend(out_names)
    if partition_name:
        in_names.append(partition_name)

    def _body(*args):
        operands = list(args)
        if partition_name:
            operands.append(partition_id_tensor())
        return tuple(_bass_exec_p.bind(
            *operands, out_avals=tuple(out_avals), in_names=tuple(in_names),
            out_names=tuple(out_names), lowering_input_output_aliases=(),
            sim_require_finite=True, sim_require_nnan=True, nc=nc))

    devices = jax.devices()[:NCORES]
    mesh = Mesh(np.asarray(devices), ("core",))
    specs = (PartitionSpec("core"),)
    sharded = jax.jit(
        shard_map(_body, mesh=mesh, in_specs=specs * (n_params + 1),
                  out_specs=specs, check_rep=False),
        donate_argnums=(n_params,), keep_unused=True)

    def call(xall):
        out, = sharded(xall.reshape(NCORES * OWN, D),
                       np.zeros((NCORES * P, 1), np.float32))
        return np.asarray(out).reshape(NCORES, P, 1)

    return call


def run(emb_i, emb_j, trace=False, **kw):
    nc = build_nc()
    fast = _NC_CACHE.get("fast_call")
    if trace or fast is None:
        res = bass_utils.run_bass_kernel_spmd(
            nc, make_in_maps(emb_i, emb_j), core_ids=list(range(NCORES)),
            trace=trace, **kw)
        partials = np.stack([r["out"] for r in res.results])  # [8,128,1]
        if fast is None:
            fast = _build_fast_call(nc)
            fast(_make_xall(emb_i, emb_j))  # warm jit trace + exec caches
            _NC_CACHE["fast_call"] = fast
    else:
        partials = fast(_make_xall(emb_i, emb_j))
        res = bass_utils.BassKernelResults(
            results=[{"out": partials[k]} for k in range(NCORES)],
            instructions_and_trace=None, profile_json=None, exec_time_ns=None)
    loss = np.float32(partials.astype(np.float64).sum() / ROWS)
    return loss, res


def kernel(emb_i, emb_j):
    loss, _ = run(emb_i, emb_j, trace=False)
    return np.asarray(loss, dtype=np.float32)


# revision 7
# speedup vs baseline: 14.8870x; 1.7869x over previous
"""Contrastive (NT-Xent) loss kernel for Trainium2, 8 NeuronCores SPMD.

Math (B=4096, D=256, T=0.5):
  z = l2norm(emb) rows; reps=[z_i; z_j] (8192 x 256); sim = reps @ reps.T
  denom_r = sum_{c != r} exp(sim[r,c]/T);  pos_m = z_i[m].z_j[m]
  loss = mean_r( ln(denom_r) - pos_r/T )

Distribution: core k receives ONLY its row shard x = [emb_i rows
[512k,512k+512); emb_j rows [512k,512k+512)] as fp16 (512KB/core instead
of a replicated 9MB) — H2D over the axon tunnel is the wall-clock
bottleneck, not device compute. Each core normalizes its 1024 rows,
transposes them to d-major fp16 tiles, and the 8 cores AllGather those
tiles HBM->HBM (512KB -> 4MB, ~15us on-chip, compute engines stay free).
The gathered column order is a core-major permutation of the reference
row order, which is harmless: the denominator is a permutation-invariant
row sum and the diagonal term is removed analytically (exp(2*||z||^2)=e^2).

Per-core main loop (8 m-tiles x 4 column groups of 2048):
  matmul fp16 -> PSUM fp32 [128,2048], ACT Exp(scale=2) in-place with
  accum_out -> per-(m,g) row partial sums; tail: ln(rowsum - e^2) minus
  4*sum(pos) -> per-partition partial [128,1] per core.
Host: loss = sum(partials)/(2B).  (gather/unshard = sum of shards)
"""

import numpy as np
from contextlib import ExitStack

import concourse.bass as bass
import concourse.tile as tile
from concourse import bacc, mybir
from concourse import bass_utils

B = 4096
D = 256
TEMP = 0.5
NCORES = 8
ROWS = 2 * B            # 8192 reps rows
PER = B // NCORES       # 512 rows of emb_i (and emb_j) per core
OWN = 2 * PER           # 1024 reps rows per core
P = 128
NG = 4                  # column groups
GCOLS = ROWS // NG      # 2048 columns per group
MT = OWN // P           # 8 m-tiles per core
NT = OWN // P           # 8 own row-tiles
F32 = mybir.dt.float32
DT = mybir.dt.float16   # matmul dtype
DTIN = mybir.dt.float8e4  # wire dtype (e4m3): halves H2D vs fp16; loss
                          # averages the ~6e-2 per-element quantization
                          # noise over 8192 rows x 8192 cols to ~1e-4
INV_T = 1.0 / TEMP      # 2.0
DIAG = float(np.exp(np.float32(INV_T), dtype=np.float32))  # exp(2*||z||^2), ||z||~1


def _kernel_body(ctx: ExitStack, tc: tile.TileContext, out_ap, x):
    nc = tc.nc
    AF = mybir.ActivationFunctionType
    ALU = mybir.AluOpType

    own_pool = ctx.enter_context(tc.tile_pool(name="own", bufs=1))
    zt_pool = ctx.enter_context(tc.tile_pool(name="zt", bufs=1))
    fin_pool = ctx.enter_context(tc.tile_pool(name="fin", bufs=1))
    ps_pool = ctx.enter_context(tc.tile_pool(name="ps", bufs=2, space="PSUM"))
    dram_pool = ctx.enter_context(tc.tile_pool(name="dram", bufs=1, space="DRAM"))

    rowparts = fin_pool.tile([P, MT * NG], F32, tag="rowparts")
    negdiag = fin_pool.tile([P, 1], F32, tag="negdiag")
    nc.gpsimd.memset(negdiag[:], -DIAG)

    # ---------------- own-block prologue ----------------
    own_x8 = own_pool.tile([P, NT, D], DTIN, tag="own_x8")  # [128,8,256]
    nc.sync.dma_start(own_x8[:], x.rearrange("(t p) d -> p t d", p=P))
    own_x = own_pool.tile([P, NT, D], DT, tag="own_x")
    nc.vector.tensor_copy(own_x[:], own_x8[:])

    # rowwise sq-sums -> inv_norm = Exp(-0.5*Ln(s))
    sq3 = own_pool.tile([P, NT, D], F32, tag="sq3")
    nc.vector.tensor_mul(sq3[:], own_x[:], own_x[:])
    sqs = own_pool.tile([P, NT], F32, tag="sqs")
    nc.vector.reduce_sum(out=sqs[:], in_=sq3[:], axis=mybir.AxisListType.X)
    inv = own_pool.tile([P, NT], F32, tag="inv")
    nc.scalar.activation(out=inv[:], in_=sqs[:], func=AF.Ln)
    nc.scalar.activation(out=inv[:], in_=inv[:], func=AF.Exp, scale=-0.5)

    z_own = own_pool.tile([P, NT, D], DT, tag="z_own")
    for t in range(NT):
        nc.vector.tensor_scalar_mul(
            out=z_own[:, t, :], in0=own_x[:, t, :], scalar1=inv[:, t:t + 1])

    # transpose own rows to d-major: zt_own[h][d, col] with d in half h
    zt_own = [own_pool.tile([P, OWN], DT, tag=f"zt_own{h}", name=f"zt_own{h}")
              for h in range(2)]
    for h in range(2):
        for t in range(NT):
            nc.sync.dma_start_transpose(
                out=zt_own[h][:, t * P:(t + 1) * P],
                in_=z_own[:, t, h * P:(h + 1) * P])

    # positives: pos_t = (x_i[t] . x_j[t]) * inv_i[t] * inv_j[t]
    nt2 = NT // 2
    pr3 = own_pool.tile([P, nt2, D], F32, tag="pr3")
    nc.vector.tensor_mul(pr3[:], own_x[:, 0:nt2, :], own_x[:, nt2:NT, :])
    pos = own_pool.tile([P, nt2], F32, tag="pos")
    nc.vector.reduce_sum(out=pos[:], in_=pr3[:], axis=mybir.AxisListType.X)
    nc.vector.tensor_mul(pos[:], pos[:], inv[:, 0:nt2])
    nc.vector.tensor_mul(pos[:], pos[:], inv[:, nt2:NT])

    # ---------------- all-gather d-major z ----------------
    cc_in = dram_pool.tile([2, P, OWN], DT, name="cc_in")
    for h in range(2):
        nc.gpsimd.dma_start(cc_in[h], zt_own[h][:])
    cc_out = dram_pool.tile([NCORES, 2, P, OWN], DT, addr_space="Shared",
                            name="cc_out")
    nc.gpsimd.collective_compute(
        "AllGather", mybir.AluOpType.bypass,
        replica_groups=[list(range(NCORES))],
        ins=[cc_in.opt()], outs=[cc_out.opt()])

    # rhs tiles: zt[g][h][:, j*OWN:(j+1)*OWN] = core (2g+j)'s half-h block
    zt = [[None, None] for _ in range(NG)]
    for g in range(NG):
        for h in range(2):
            zt[g][h] = zt_pool.tile([P, GCOLS], DT, tag=f"zt{g}_{h}",
                                    name=f"zt{g}_{h}")
            for j in range(2):
                nc.sync.dma_start(zt[g][h][:, j * OWN:(j + 1) * OWN],
                                  cc_out[2 * g + j, h])

    # ---------------- main loop ----------------
    def main_unit(g, m):
        ps = ps_pool.tile([P, GCOLS], F32, tag="ps", name="ps")
        nsub = GCOLS // 512
        for h in range(2):
            for ns in range(nsub):
                nc.tensor.matmul(
                    ps[:, ns * 512:(ns + 1) * 512],
                    lhsT=zt_own[h][:, m * P:(m + 1) * P],
                    rhs=zt[g][h][:, ns * 512:(ns + 1) * 512],
                    start=(h == 0), stop=(h == 1))
        nc.scalar.activation(
            out=ps[:], in_=ps[:], func=AF.Exp, scale=INV_T,
            accum_out=rowparts[:, m * NG + g: m * NG + g + 1])

    for g in range(NG):
        for m in range(MT):
            main_unit(g, m)

    # ---------------- tail ----------------
    denom = fin_pool.tile([P, MT], F32, tag="denom")
    nc.vector.reduce_sum(
        out=denom[:], in_=rowparts[:].rearrange("p (m g) -> p m g", g=NG),
        axis=mybir.AxisListType.X)
    ln8 = fin_pool.tile([P, MT], F32, tag="ln8")
    nc.scalar.activation(out=ln8[:], in_=denom[:], func=AF.Ln, bias=negdiag[:])
    lnsum = fin_pool.tile([P, 1], F32, tag="lnsum")
    nc.vector.reduce_sum(out=lnsum[:], in_=ln8[:], axis=mybir.AxisListType.X)
    possum = fin_pool.tile([P, 1], F32, tag="possum")
    nc.vector.reduce_sum(out=possum[:], in_=pos[:], axis=mybir.AxisListType.X)
    partial = fin_pool.tile([P, 1], F32, tag="partial")
    # partial = lnsum - 2*INV_T*possum   (each pos appears for a z_i and a z_j row)
    nc.vector.tensor_scalar(
        out=partial[:], in0=possum[:], scalar1=-2.0 * INV_T, scalar2=lnsum[:],
        op0=ALU.mult, op1=ALU.add)
    nc.sync.dma_start(out_ap, partial[:])


_NC_CACHE = {}


def build_nc():
    if "nc" in _NC_CACHE:
        return _NC_CACHE["nc"]
    nc = bacc.Bacc("TRN2", target_bir_lowering=False, debug=False,
                   enable_asserts=False, num_devices=NCORES)
    x = nc.dram_tensor("x", (OWN, D), DTIN, kind="ExternalInput").ap()
    out = nc.dram_tensor("out", (P, 1), F32, kind="ExternalOutput").ap()
    with tile.TileContext(nc) as tc:
        with ExitStack() as ctx:
            _kernel_body(ctx, tc, out, x)
    nc.compile()
    _NC_CACHE["nc"] = nc
    return nc


def _make_xall(emb_i, emb_j):
    npin = mybir.dt.np(DTIN)
    xi = np.asarray(emb_i).astype(npin).reshape(NCORES, PER, D)
    xj = np.asarray(emb_j).astype(npin).reshape(NCORES, PER, D)
    return np.concatenate([xi, xj], axis=1)  # [8, 1024, 256]; [k] = core k shard


def make_in_maps(emb_i, emb_j):
    xall = _make_xall(emb_i, emb_j)
    return [{"x": xall[k]} for k in range(NCORES)]


def _build_fast_call(nc):
    """Cached-dispatch twin of bass_utils.run_bass_kernel_spmd's axon/PJRT
    exec step: same NEFF, same shard_map over cores 0-7, but the jitted
    callable is built once and reused, instead of a fresh closure (and a
    ~0.15s jax re-trace) per call."""
    import jax
    from concourse.bass2jax import (_bass_exec_p, install_neuronx_cc_hook,
                                    partition_id_tensor)
    from jax.sharding import Mesh, PartitionSpec
    from jax.experimental.shard_map import shard_map

    install_neuronx_cc_hook()
    partition_name = (nc.partition_id_tensor.name
                      if nc.partition_id_tensor else None)
    in_names, out_names, out_avals = [], [], []
    for alloc in nc.m.functions[0].allocations:
        if not isinstance(alloc, mybir.MemoryLocationSet):
            continue
        name = alloc.memorylocations[0].name
        if alloc.kind == "ExternalInput":
            if name != partition_name:
                in_names.append(name)
        elif alloc.kind == "ExternalOutput":
            out_names.append(name)
            shape = tuple(alloc.tensor_shape)
            out_avals.append(jax.core.ShapedArray(shape, mybir.dt.np(alloc.dtype)))
    assert in_names == ["x"] and out_names == ["out"]
    n_params = len(in_names)
    in_names.extend(out_names)
    if partition_name:
        in_names.append(partition_name)

    def _body(*args):
        operands = list(args)
        if partition_name:
            operands.append(partition_id_tensor())
        return tuple(_bass_exec_p.bind(
            *operands, out_avals=tuple(out_avals), in_names=tuple(in_names),
            out_names=tuple(out_names), lowering_input_output_aliases=(),
            sim_require_finite=True, sim_require_nnan=True, nc=nc))

    devices = jax.devices()[:NCORES]
    mesh = Mesh(np.asarray(devices), ("core",))
    specs = (PartitionSpec("core"),)
    sharded = jax.jit(
        shard_map(_body, mesh=mesh, in_specs=specs * (n_params + 1),
                  out_specs=specs, check_rep=False),
        donate_argnums=(n_params,), keep_unused=True)

    def call(xall):
        out, = sharded(xall.reshape(NCORES * OWN, D),
                       np.zeros((NCORES * P, 1), np.float32))
        return np.asarray(out).reshape(NCORES, P, 1)

    return call


def run(emb_i, emb_j, trace=False, **kw):
    nc = build_nc()
    fast = _NC_CACHE.get("fast_call")
    if trace or fast is None:
        res = bass_utils.run_bass_kernel_spmd(
            nc, make_in_maps(emb_i, emb_j), core_ids=list(range(NCORES)),
            trace=trace, **kw)
        partials = np.stack([r["out"] for r in res.results])  # [8,128,1]
        if fast is None:
            fast = _build_fast_call(nc)
            fast(_make_xall(emb_i, emb_j))  # warm jit trace + exec caches
            _NC_CACHE["fast_call"] = fast
    else:
        partials = fast(_make_xall(emb_i, emb_j))
        res = bass_utils.BassKernelResults(
            results=[{"out": partials[k]} for k in range(NCORES)],
            instructions_and_trace=None, profile_json=None, exec_time_ns=None)
    loss = np.float32(partials.astype(np.float64).sum() / ROWS)
    return loss, res


def kernel(emb_i, emb_j):
    loss, _ = run(emb_i, emb_j, trace=False)
    return np.asarray(loss, dtype=np.float32)


# revision 10
# speedup vs baseline: 6243.0749x; 419.3635x over previous
"""Contrastive (NT-Xent) loss kernel for Trainium2, 8 NeuronCores SPMD.

Math (B=4096, D=256, T=0.5):
  z = l2norm(emb) rows; reps=[z_i; z_j] (8192 x 256); sim = reps @ reps.T
  denom_r = sum_{c != r} exp(sim[r,c]/T);  pos_m = z_i[m].z_j[m]
  loss = mean_r( ln(denom_r) - pos_r/T )

Distribution: core k receives ONLY its row shard x = [emb_i rows
[512k,512k+512); emb_j rows [512k,512k+512)] as fp16 (512KB/core instead
of a replicated 9MB) — H2D over the axon tunnel is the wall-clock
bottleneck, not device compute. Each core normalizes its 1024 rows,
transposes them to d-major fp16 tiles, and the 8 cores AllGather those
tiles HBM->HBM (512KB -> 4MB, ~15us on-chip, compute engines stay free).
The gathered column order is a core-major permutation of the reference
row order, which is harmless: the denominator is a permutation-invariant
row sum and the diagonal term is removed analytically (exp(2*||z||^2)=e^2).

Per-core main loop (8 m-tiles x 4 column groups of 2048):
  matmul fp16 -> PSUM fp32 [128,2048], ACT Exp(scale=2) in-place with
  accum_out -> per-(m,g) row partial sums; tail: ln(rowsum - e^2) minus
  4*sum(pos) -> per-partition partial [128,1] per core.
Host: loss = sum(partials)/(2B).  (gather/unshard = sum of shards)
"""

import numpy as np
from contextlib import ExitStack

import concourse.bass as bass
import concourse.tile as tile
from concourse import bacc, mybir
from concourse import bass_utils

B = 4096
D = 256
TEMP = 0.5
NCORES = 8
ROWS = 2 * B            # 8192 reps rows
PER = B // NCORES       # 512 rows of emb_i (and emb_j) per core
OWN = 2 * PER           # 1024 reps rows per core
P = 128
NG = 4                  # column groups
GCOLS = ROWS // NG      # 2048 columns per group
MT = OWN // P           # 8 m-tiles per core
NT = OWN // P           # 8 own row-tiles
F32 = mybir.dt.float32
DT = mybir.dt.float16   # matmul dtype
DTIN = mybir.dt.float8e4  # wire dtype (e4m3): halves H2D vs fp16; loss
                          # averages the ~6e-2 per-element quantization
                          # noise over 8192 rows x 8192 cols to ~1e-4
INV_T = 1.0 / TEMP      # 2.0
DIAG = float(np.exp(np.float32(INV_T), dtype=np.float32))  # exp(2*||z||^2), ||z||~1


def _kernel_body(ctx: ExitStack, tc: tile.TileContext, out_ap, x):
    nc = tc.nc
    AF = mybir.ActivationFunctionType
    ALU = mybir.AluOpType

    own_pool = ctx.enter_context(tc.tile_pool(name="own", bufs=1))
    zt_pool = ctx.enter_context(tc.tile_pool(name="zt", bufs=1))
    fin_pool = ctx.enter_context(tc.tile_pool(name="fin", bufs=1))
    ps_pool = ctx.enter_context(tc.tile_pool(name="ps", bufs=2, space="PSUM"))
    dram_pool = ctx.enter_context(tc.tile_pool(name="dram", bufs=1, space="DRAM"))

    rowparts = fin_pool.tile([P, MT * NG], F32, tag="rowparts")
    negdiag = fin_pool.tile([P, 1], F32, tag="negdiag")
    nc.gpsimd.memset(negdiag[:], -DIAG)

    # ---------------- own-block prologue ----------------
    own_x8 = own_pool.tile([P, NT, D], DTIN, tag="own_x8")  # [128,8,256]
    nc.sync.dma_start(own_x8[:], x.rearrange("(t p) d -> p t d", p=P))
    own_x = own_pool.tile([P, NT, D], DT, tag="own_x")
    nc.vector.tensor_copy(own_x[:], own_x8[:])

    # rowwise sq-sums -> inv_norm = Exp(-0.5*Ln(s))
    sq3 = own_pool.tile([P, NT, D], F32, tag="sq3")
    nc.vector.tensor_mul(sq3[:], own_x[:], own_x[:])
    sqs = own_pool.tile([P, NT], F32, tag="sqs")
    nc.vector.reduce_sum(out=sqs[:], in_=sq3[:], axis=mybir.AxisListType.X)
    inv = own_pool.tile([P, NT], F32, tag="inv")
    nc.scalar.activation(out=inv[:], in_=sqs[:], func=AF.Ln)
    nc.scalar.activation(out=inv[:], in_=inv[:], func=AF.Exp, scale=-0.5)

    z_own = own_pool.tile([P, NT, D], DT, tag="z_own")
    for t in range(NT):
        nc.vector.tensor_scalar_mul(
            out=z_own[:, t, :], in0=own_x[:, t, :], scalar1=inv[:, t:t + 1])

    # transpose own rows to d-major: zt_own[h][d, col] with d in half h.
    # fp16 transposes (xbar needs 2-byte) alternating over both HWDGE
    # queues (SP + ACT), then a DVE cast to fp8 per half feeds the
    # collective with half the wire bytes; matmul also runs fp8 (2x PE).
    zt_own = [own_pool.tile([P, OWN], DT, tag=f"zt_own{h}", name=f"zt_own{h}")
              for h in range(2)]
    zt_own8 = [own_pool.tile([P, OWN], DTIN, tag=f"zt_own8{h}",
                             name=f"zt_own8{h}") for h in range(2)]
    cc_in = dram_pool.tile([2, P, OWN], DTIN, name="cc_in")
    for h in range(2):
        for t in range(NT):
            eng = nc.sync if t % 2 == 0 else nc.scalar
            eng.dma_start_transpose(
                out=zt_own[h][:, t * P:(t + 1) * P],
                in_=z_own[:, t, h * P:(h + 1) * P])
        nc.vector.tensor_copy(zt_own8[h][:], zt_own[h][:])
        nc.gpsimd.dma_start(cc_in[h], zt_own8[h][:])

    # ---------------- all-gather d-major z (fp8) ----------------
    cc_out = dram_pool.tile([NCORES, 2, P, OWN], DTIN, addr_space="Shared",
                            name="cc_out")
    nc.gpsimd.collective_compute(
        "AllGather", mybir.AluOpType.bypass,
        replica_groups=[list(range(NCORES))],
        ins=[cc_in.opt()], outs=[cc_out.opt()])

    # positives: pos_t = (x_i[t] . x_j[t]) * inv_i[t] * inv_j[t]
    # (issued after the collective trigger so DVE work hides in its shadow)
    nt2 = NT // 2
    pr3 = own_pool.tile([P, nt2, D], F32, tag="pr3")
    nc.vector.tensor_mul(pr3[:], own_x[:, 0:nt2, :], own_x[:, nt2:NT, :])
    pos = own_pool.tile([P, nt2], F32, tag="pos")
    nc.vector.reduce_sum(out=pos[:], in_=pr3[:], axis=mybir.AxisListType.X)
    nc.vector.tensor_mul(pos[:], pos[:], inv[:, 0:nt2])
    nc.vector.tensor_mul(pos[:], pos[:], inv[:, nt2:NT])

    # rhs tiles: zt[g][h][:, j*OWN:(j+1)*OWN] = core (2g+j)'s half-h block
    zt = [[None, None] for _ in range(NG)]
    for g in range(NG):
        for h in range(2):
            zt[g][h] = zt_pool.tile([P, GCOLS], DTIN, tag=f"zt{g}_{h}",
                                    name=f"zt{g}_{h}")
            for j in range(2):
                eng = nc.sync if (h + j) % 2 == 0 else nc.scalar
                eng.dma_start(zt[g][h][:, j * OWN:(j + 1) * OWN],
                              cc_out[2 * g + j, h])

    # ---------------- main loop ----------------
    def main_unit(g, m):
        ps = ps_pool.tile([P, GCOLS], F32, tag="ps", name="ps")
        nsub = GCOLS // 512
        for h in range(2):
            for ns in range(nsub):
                nc.tensor.matmul(
                    ps[:, ns * 512:(ns + 1) * 512],
                    lhsT=zt_own8[h][:, m * P:(m + 1) * P],
                    rhs=zt[g][h][:, ns * 512:(ns + 1) * 512],
                    start=(h == 0), stop=(h == 1))
        nc.scalar.activation(
            out=ps[:], in_=ps[:], func=AF.Exp, scale=INV_T,
            accum_out=rowparts[:, m * NG + g: m * NG + g + 1])

    for g in range(NG):
        for m in range(MT):
            main_unit(g, m)

    # ---------------- tail ----------------
    denom = fin_pool.tile([P, MT], F32, tag="denom")
    nc.vector.reduce_sum(
        out=denom[:], in_=rowparts[:].rearrange("p (m g) -> p m g", g=NG),
        axis=mybir.AxisListType.X)
    ln8 = fin_pool.tile([P, MT], F32, tag="ln8")
    nc.scalar.activation(out=ln8[:], in_=denom[:], func=AF.Ln, bias=negdiag[:])
    lnsum = fin_pool.tile([P, 1], F32, tag="lnsum")
    nc.vector.reduce_sum(out=lnsum[:], in_=ln8[:], axis=mybir.AxisListType.X)
    possum = fin_pool.tile([P, 1], F32, tag="possum")
    nc.vector.reduce_sum(out=possum[:], in_=pos[:], axis=mybir.AxisListType.X)
    partial = fin_pool.tile([P, 1], F32, tag="partial")
    # partial = lnsum - 2*INV_T*possum   (each pos appears for a z_i and a z_j row)
    nc.vector.tensor_scalar(
        out=partial[:], in0=possum[:], scalar1=-2.0 * INV_T, scalar2=lnsum[:],
        op0=ALU.mult, op1=ALU.add)
    nc.sync.dma_start(out_ap, partial[:])


_NC_CACHE = {}


def build_nc():
    if "nc" in _NC_CACHE:
        return _NC_CACHE["nc"]
    nc = bacc.Bacc("TRN2", target_bir_lowering=False, debug=False,
                   enable_asserts=False, num_devices=NCORES)
    x = nc.dram_tensor("x", (OWN, D), DTIN, kind="ExternalInput").ap()
    out = nc.dram_tensor("out", (P, 1), F32, kind="ExternalOutput").ap()
    with tile.TileContext(nc) as tc:
        with ExitStack() as ctx:
            _kernel_body(ctx, tc, out, x)
    nc.compile()
    _NC_CACHE["nc"] = nc
    return nc


def _make_xall(emb_i, emb_j):
    npin = mybir.dt.np(DTIN)
    xi = np.asarray(emb_i).astype(npin).reshape(NCORES, PER, D)
    xj = np.asarray(emb_j).astype(npin).reshape(NCORES, PER, D)
    return np.concatenate([xi, xj], axis=1)  # [8, 1024, 256]; [k] = core k shard


def make_in_maps(emb_i, emb_j):
    xall = _make_xall(emb_i, emb_j)
    return [{"x": xall[k]} for k in range(NCORES)]


def _build_fast_call(nc):
    """Cached-dispatch twin of bass_utils.run_bass_kernel_spmd's axon/PJRT
    exec step: same NEFF, same shard_map over cores 0-7, but the jitted
    callable is built once and reused, instead of a fresh closure (and a
    ~0.15s jax re-trace) per call."""
    import jax
    from concourse.bass2jax import (_bass_exec_p, install_neuronx_cc_hook,
                                    partition_id_tensor)
    from jax.sharding import Mesh, PartitionSpec
    from jax.experimental.shard_map import shard_map

    install_neuronx_cc_hook()
    partition_name = (nc.partition_id_tensor.name
                      if nc.partition_id_tensor else None)
    in_names, out_names, out_avals = [], [], []
    for alloc in nc.m.functions[0].allocations:
        if not isinstance(alloc, mybir.MemoryLocationSet):
            continue
        name = alloc.memorylocations[0].name
        if alloc.kind == "ExternalInput":
            if name != partition_name:
                in_names.append(name)
        elif alloc.kind == "ExternalOutput":
            out_names.append(name)
            shape = tuple(alloc.tensor_shape)
            out_avals.append(jax.core.ShapedArray(shape, mybir.dt.np(alloc.dtype)))
    assert in_names == ["x"] and out_names == ["out"]
    n_params = len(in_names)
    in_names.extend(out_names)
    if partition_name:
        in_names.append(partition_name)

    def _body(*args):
        operands = list(args)
        if partition_name:
            operands.append(partition_id_tensor())
        return tuple(_bass_exec_p.bind(
            *operands, out_avals=tuple(out_avals), in_names=tuple(in_names),
            out_names=tuple(out_names), lowering_input_output_aliases=(),
            sim_require_finite=True, sim_require_nnan=True, nc=nc))

    devices = jax.devices()[:NCORES]
    mesh = Mesh(np.asarray(devices), ("core",))
    specs = (PartitionSpec("core"),)
    sharded = jax.jit(
        shard_map(_body, mesh=mesh, in_specs=specs * (n_params + 1),
                  out_specs=specs, check_rep=False),
        donate_argnums=(n_params,), keep_unused=True)

    def call(xall):
        out, = sharded(xall.reshape(NCORES * OWN, D),
                       np.zeros((NCORES * P, 1), np.float32))
        return np.asarray(out).reshape(NCORES, P, 1)

    return call


def run(emb_i, emb_j, trace=False, **kw):
    nc = build_nc()
    fast = _NC_CACHE.get("fast_call")
    if trace or fast is None:
        res = bass_utils.run_bass_kernel_spmd(
            nc, make_in_maps(emb_i, emb_j), core_ids=list(range(NCORES)),
            trace=trace, **kw)
        partials = np.stack([r["out"] for r in res.results])  # [8,128,1]
        if fast is None:
            fast = _build_fast_call(nc)
            fast(_make_xall(emb_i, emb_j))  # warm jit trace + exec caches
            _NC_CACHE["fast_call"] = fast
    else:
        partials = fast(_make_xall(emb_i, emb_j))
        res = bass_utils.BassKernelResults(
            results=[{"out": partials[k]} for k in range(NCORES)],
            instructions_and_trace=None, profile_json=None, exec_time_ns=None)
    loss = np.float32(partials.astype(np.float64).sum() / ROWS)
    return loss, res


def kernel(emb_i, emb_j):
    loss, _ = run(emb_i, emb_j, trace=False)
    return np.asarray(loss, dtype=np.float32)


# revision 11
# speedup vs baseline: 6996.5286x; 1.1207x over previous
"""Contrastive (NT-Xent) loss kernel for Trainium2, 8 NeuronCores SPMD.

Math (B=4096, D=256, T=0.5):
  z = l2norm(emb) rows; reps=[z_i; z_j] (8192 x 256); sim = reps @ reps.T
  denom_r = sum_{c != r} exp(sim[r,c]/T);  pos_m = z_i[m].z_j[m]
  loss = mean_r( ln(denom_r) - pos_r/T )

Distribution: core k receives ONLY its row shard x = [emb_i rows
[512k,512k+512); emb_j rows [512k,512k+512)] as fp16 (512KB/core instead
of a replicated 9MB) — H2D over the axon tunnel is the wall-clock
bottleneck, not device compute. Each core normalizes its 1024 rows,
transposes them to d-major fp16 tiles, and the 8 cores AllGather those
tiles HBM->HBM (512KB -> 4MB, ~15us on-chip, compute engines stay free).
The gathered column order is a core-major permutation of the reference
row order, which is harmless: the denominator is a permutation-invariant
row sum and the diagonal term is removed analytically (exp(2*||z||^2)=e^2).

Per-core main loop (8 m-tiles x 4 column groups of 2048):
  matmul fp16 -> PSUM fp32 [128,2048], ACT Exp(scale=2) in-place with
  accum_out -> per-(m,g) row partial sums; tail: ln(rowsum - e^2) minus
  4*sum(pos) -> per-partition partial [128,1] per core.
Host: loss = sum(partials)/(2B).  (gather/unshard = sum of shards)
"""

import numpy as np
from contextlib import ExitStack

import concourse.bass as bass
import concourse.tile as tile
from concourse import bacc, mybir
from concourse import bass_utils

B = 4096
D = 256
TEMP = 0.5
NCORES = 8
ROWS = 2 * B            # 8192 reps rows
PER = B // NCORES       # 512 rows of emb_i (and emb_j) per core
OWN = 2 * PER           # 1024 reps rows per core
P = 128
NG = 4                  # column groups
GCOLS = ROWS // NG      # 2048 columns per group
MT = OWN // P           # 8 m-tiles per core
NT = OWN // P           # 8 own row-tiles
F32 = mybir.dt.float32
DT = mybir.dt.float16   # matmul dtype
DTIN = mybir.dt.float8e4  # wire dtype (e4m3): halves H2D vs fp16; loss
                          # averages the ~6e-2 per-element quantization
                          # noise over 8192 rows x 8192 cols to ~1e-4
INV_T = 1.0 / TEMP      # 2.0
DIAG = float(np.exp(np.float32(INV_T), dtype=np.float32))  # exp(2*||z||^2), ||z||~1


def _kernel_body(ctx: ExitStack, tc: tile.TileContext, out_ap, x):
    nc = tc.nc
    AF = mybir.ActivationFunctionType
    ALU = mybir.AluOpType

    own_pool = ctx.enter_context(tc.tile_pool(name="own", bufs=1))
    zt_pool = ctx.enter_context(tc.tile_pool(name="zt", bufs=1))
    fin_pool = ctx.enter_context(tc.tile_pool(name="fin", bufs=1))
    ps_pool = ctx.enter_context(tc.tile_pool(name="ps", bufs=2, space="PSUM"))
    dram_pool = ctx.enter_context(tc.tile_pool(name="dram", bufs=1, space="DRAM"))

    rowparts = fin_pool.tile([P, MT * NG], F32, tag="rowparts")
    negdiag = fin_pool.tile([P, 1], F32, tag="negdiag")
    nc.gpsimd.memset(negdiag[:], -DIAG)

    # ---------------- own-block prologue ----------------
    # per-128-row-tile pipeline: load t -> cast t -> sq t -> reduce t, so
    # the norm chain streams behind the DMA instead of waiting for the
    # whole 256KB strided load (the collective trigger is downstream of
    # all of this, and every core's trigger time gates the rendezvous)
    xr = x.rearrange("(t p) d -> p t d", p=P)
    own_x8 = own_pool.tile([P, NT, D], DTIN, tag="own_x8")  # [128,8,256]
    own_x = own_pool.tile([P, NT, D], DT, tag="own_x")
    sq3 = own_pool.tile([P, NT, D], F32, tag="sq3")
    sqs = own_pool.tile([P, NT], F32, tag="sqs")
    for t in range(NT):
        eng = nc.sync if t % 2 == 0 else nc.scalar
        eng.dma_start(own_x8[:, t, :], xr[:, t, :])
        nc.vector.tensor_copy(own_x[:, t, :], own_x8[:, t, :])
        nc.vector.tensor_mul(sq3[:, t, :], own_x[:, t, :], own_x[:, t, :])
        nc.vector.reduce_sum(out=sqs[:, t:t + 1], in_=sq3[:, t, :],
                             axis=mybir.AxisListType.X)
    inv = own_pool.tile([P, NT], F32, tag="inv")
    nc.scalar.activation(out=inv[:], in_=sqs[:], func=AF.Ln)
    nc.scalar.activation(out=inv[:], in_=inv[:], func=AF.Exp, scale=-0.5)

    z_own = own_pool.tile([P, NT, D], DT, tag="z_own")
    for t in range(NT):
        nc.vector.tensor_scalar_mul(
            out=z_own[:, t, :], in0=own_x[:, t, :], scalar1=inv[:, t:t + 1])

    # transpose own rows to d-major: zt_own[h][d, col] with d in half h.
    # fp16 transposes (xbar needs 2-byte) alternating over both HWDGE
    # queues (SP + ACT), then a DVE cast to fp8 per half feeds the
    # collective with half the wire bytes; matmul also runs fp8 (2x PE).
    zt_own = [own_pool.tile([P, OWN], DT, tag=f"zt_own{h}", name=f"zt_own{h}")
              for h in range(2)]
    zt_own8 = [own_pool.tile([P, OWN], DTIN, tag=f"zt_own8{h}",
                             name=f"zt_own8{h}") for h in range(2)]
    cc_in = dram_pool.tile([2, P, OWN], DTIN, name="cc_in")
    for h in range(2):
        for t in range(NT):
            eng = nc.sync if t % 2 == 0 else nc.scalar
            eng.dma_start_transpose(
                out=zt_own[h][:, t * P:(t + 1) * P],
                in_=z_own[:, t, h * P:(h + 1) * P])
        nc.vector.tensor_copy(zt_own8[h][:], zt_own[h][:])
        nc.gpsimd.dma_start(cc_in[h], zt_own8[h][:])

    # ---------------- all-gather d-major z (fp8) ----------------
    cc_out = dram_pool.tile([NCORES, 2, P, OWN], DTIN, addr_space="Shared",
                            name="cc_out")
    nc.gpsimd.collective_compute(
        "AllGather", mybir.AluOpType.bypass,
        replica_groups=[list(range(NCORES))],
        ins=[cc_in.opt()], outs=[cc_out.opt()])

    # positives: pos_t = (x_i[t] . x_j[t]) * inv_i[t] * inv_j[t]
    # (issued after the collective trigger so DVE work hides in its shadow)
    nt2 = NT // 2
    pr3 = own_pool.tile([P, nt2, D], F32, tag="pr3")
    nc.vector.tensor_mul(pr3[:], own_x[:, 0:nt2, :], own_x[:, nt2:NT, :])
    pos = own_pool.tile([P, nt2], F32, tag="pos")
    nc.vector.reduce_sum(out=pos[:], in_=pr3[:], axis=mybir.AxisListType.X)
    nc.vector.tensor_mul(pos[:], pos[:], inv[:, 0:nt2])
    nc.vector.tensor_mul(pos[:], pos[:], inv[:, nt2:NT])

    # rhs tiles: zt[g][h][:, j*OWN:(j+1)*OWN] = core (2g+j)'s half-h block
    zt = [[None, None] for _ in range(NG)]
    for g in range(NG):
        for h in range(2):
            zt[g][h] = zt_pool.tile([P, GCOLS], DTIN, tag=f"zt{g}_{h}",
                                    name=f"zt{g}_{h}")
            for j in range(2):
                eng = nc.sync if (h + j) % 2 == 0 else nc.scalar
                eng.dma_start(zt[g][h][:, j * OWN:(j + 1) * OWN],
                              cc_out[2 * g + j, h])

    # ---------------- main loop ----------------
    def main_unit(g, m):
        ps = ps_pool.tile([P, GCOLS], F32, tag="ps", name="ps")
        nsub = GCOLS // 512
        for h in range(2):
            for ns in range(nsub):
                nc.tensor.matmul(
                    ps[:, ns * 512:(ns + 1) * 512],
                    lhsT=zt_own8[h][:, m * P:(m + 1) * P],
                    rhs=zt[g][h][:, ns * 512:(ns + 1) * 512],
                    start=(h == 0), stop=(h == 1))
        nc.scalar.activation(
            out=ps[:], in_=ps[:], func=AF.Exp, scale=INV_T,
            accum_out=rowparts[:, m * NG + g: m * NG + g + 1])

    for g in range(NG):
        for m in range(MT):
            main_unit(g, m)

    # ---------------- tail ----------------
    denom = fin_pool.tile([P, MT], F32, tag="denom")
    nc.vector.reduce_sum(
        out=denom[:], in_=rowparts[:].rearrange("p (m g) -> p m g", g=NG),
        axis=mybir.AxisListType.X)
    ln8 = fin_pool.tile([P, MT], F32, tag="ln8")
    nc.scalar.activation(out=ln8[:], in_=denom[:], func=AF.Ln, bias=negdiag[:])
    lnsum = fin_pool.tile([P, 1], F32, tag="lnsum")
    nc.vector.reduce_sum(out=lnsum[:], in_=ln8[:], axis=mybir.AxisListType.X)
    possum = fin_pool.tile([P, 1], F32, tag="possum")
    nc.vector.reduce_sum(out=possum[:], in_=pos[:], axis=mybir.AxisListType.X)
    partial = fin_pool.tile([P, 1], F32, tag="partial")
    # partial = lnsum - 2*INV_T*possum   (each pos appears for a z_i and a z_j row)
    nc.vector.tensor_scalar(
        out=partial[:], in0=possum[:], scalar1=-2.0 * INV_T, scalar2=lnsum[:],
        op0=ALU.mult, op1=ALU.add)
    nc.sync.dma_start(out_ap, partial[:])


_NC_CACHE = {}


def build_nc():
    if "nc" in _NC_CACHE:
        return _NC_CACHE["nc"]
    nc = bacc.Bacc("TRN2", target_bir_lowering=False, debug=False,
                   enable_asserts=False, num_devices=NCORES)
    x = nc.dram_tensor("x", (OWN, D), DTIN, kind="ExternalInput").ap()
    out = nc.dram_tensor("out", (P, 1), F32, kind="ExternalOutput").ap()
    with tile.TileContext(nc) as tc:
        with ExitStack() as ctx:
            _kernel_body(ctx, tc, out, x)
    nc.compile()
    _NC_CACHE["nc"] = nc
    return nc


def _make_xall(emb_i, emb_j):
    npin = mybir.dt.np(DTIN)
    xi = np.asarray(emb_i).astype(npin).reshape(NCORES, PER, D)
    xj = np.asarray(emb_j).astype(npin).reshape(NCORES, PER, D)
    return np.concatenate([xi, xj], axis=1)  # [8, 1024, 256]; [k] = core k shard


def make_in_maps(emb_i, emb_j):
    xall = _make_xall(emb_i, emb_j)
    return [{"x": xall[k]} for k in range(NCORES)]


def _build_fast_call(nc):
    """Cached-dispatch twin of bass_utils.run_bass_kernel_spmd's axon/PJRT
    exec step: same NEFF, same shard_map over cores 0-7, but the jitted
    callable is built once and reused, instead of a fresh closure (and a
    ~0.15s jax re-trace) per call."""
    import jax
    from concourse.bass2jax import (_bass_exec_p, install_neuronx_cc_hook,
                                    partition_id_tensor)
    from jax.sharding import Mesh, PartitionSpec
    from jax.experimental.shard_map import shard_map

    install_neuronx_cc_hook()
    partition_name = (nc.partition_id_tensor.name
                      if nc.partition_id_tensor else None)
    in_names, out_names, out_avals = [], [], []
    for alloc in nc.m.functions[0].allocations:
        if not isinstance(alloc, mybir.MemoryLocationSet):
            continue
        name = alloc.memorylocations[0].name
        if alloc.kind == "ExternalInput":
            if name != partition_name:
                in_names.append(name)
        elif alloc.kind == "ExternalOutput":
            out_names.append(name)
            shape = tuple(alloc.tensor_shape)
            out_avals.append(jax.core.ShapedArray(shape, mybir.dt.np(alloc.dtype)))
    assert in_names == ["x"] and out_names == ["out"]
    n_params = len(in_names)
    in_names.extend(out_names)
    if partition_name:
        in_names.append(partition_name)

    def _body(*args):
        operands = list(args)
        if partition_name:
            operands.append(partition_id_tensor())
        return tuple(_bass_exec_p.bind(
            *operands, out_avals=tuple(out_avals), in_names=tuple(in_names),
            out_names=tuple(out_names), lowering_input_output_aliases=(),
            sim_require_finite=True, sim_require_nnan=True, nc=nc))

    devices = jax.devices()[:NCORES]
    mesh = Mesh(np.asarray(devices), ("core",))
    specs = (PartitionSpec("core"),)
    sharded = jax.jit(
        shard_map(_body, mesh=mesh, in_specs=specs * (n_params + 1),
                  out_specs=specs, check_rep=False),
        donate_argnums=(n_params,), keep_unused=True)

    def call(xall):
        out, = sharded(xall.reshape(NCORES * OWN, D),
                       np.zeros((NCORES * P, 1), np.float32))
        return np.asarray(out).reshape(NCORES, P, 1)

    return call


def run(emb_i, emb_j, trace=False, **kw):
    nc = build_nc()
    fast = _NC_CACHE.get("fast_call")
    if trace or fast is None:
        res = bass_utils.run_bass_kernel_spmd(
            nc, make_in_maps(emb_i, emb_j), core_ids=list(range(NCORES)),
            trace=trace, **kw)
        partials = np.stack([r["out"] for r in res.results])  # [8,128,1]
        if fast is None:
            fast = _build_fast_call(nc)
            fast(_make_xall(emb_i, emb_j))  # warm jit trace + exec caches
            _NC_CACHE["fast_call"] = fast
    else:
        partials = fast(_make_xall(emb_i, emb_j))
        res = bass_utils.BassKernelResults(
            results=[{"out": partials[k]} for k in range(NCORES)],
            instructions_and_trace=None, profile_json=None, exec_time_ns=None)
    loss = np.float32(partials.astype(np.float64).sum() / ROWS)
    return loss, res


def kernel(emb_i, emb_j):
    loss, _ = run(emb_i, emb_j, trace=False)
    return np.asarray(loss, dtype=np.float32)
